# revision 3
# baseline (speedup 1.0000x reference)
"""Talking-heads causal attention (nn_Attention_37744172597821) on 8 TRN2 cores.

Sharding: core c = (batch bb = c//2, parity r = c%2). Each core handles batch bb
and query rows {i : i % 2 == r} (512 rows, perfectly causal-balanced and
identical control flow across cores -> one SPMD program).

Device-internal conventions (per core):
  - j' axis (keys-first): j' in [0,1024) = key t ; j' in [1024,1028) = memory slot.
    Host remaps to the reference order [mem | keys] at assembly.
  - i_loc in [0,512): global query i = 2*i_loc + r. Two 256-col blocks bi=0,1.
  - j tiles: jb in 0..8 key blocks of 128 (j' = 128*jb + 8*jhi + jl) + mem tile.
  - "mix layout": partition p = jl*16 + h  (8 j's x 16 heads) -> talking-heads
    mixing becomes a 128x128 block-diagonal matmul (kron(I8, proj)).
  - DRAM intermediates (preT2 = pre-softmax dots, expm2 = unnormalized exp,
    attn2d = post-mixed attn) are stored [j', h, i_loc] so every transfer is a
    <=3-dim single-partition-dim DMA. post map = expm2/den is divided on host.
"""

import sys

sys.path.insert(0, "/opt/trn_rl_repo")

import numpy as np
import ml_dtypes

import concourse.bass as bass  # noqa: F401
import concourse.mybir as mybir
import concourse.tile as tile
from concourse import bacc
from concourse.bass_utils import run_bass_kernel_spmd

DT = mybir.dt
F32 = DT.float32
F32R = DT.float32r
BF16 = DT.bfloat16
AF = mybir.ActivationFunctionType
ALU = mybir.AluOpType

B, N, DIM, H, D, M = 4, 1024, 1024, 16, 64, 4
P = 128
NJ = N + M  # 1028
NI = 512    # i_loc per core
NEG = -1.0e5


def _classify(jb, jhi, bi):
    """(kind, c0, cH) for tile (jb,jhi) vs i-block bi. r-independent."""
    if jb == 8:
        return ("FV", 0, 0)
    K = 128 * jb + 8 * jhi - 512 * bi
    if K <= -8:
        return ("FV", 0, 0)
    if K >= 512:
        return ("FM", None, None)
    c0 = K // 2
    return ("DG", c0, min(c0 + 4, 256))


def _vis_rows(jb, bi):
    """Number of j-rows of block jb with any visible column for i-block bi."""
    if jb == 8:
        return 4
    nvis_jhi = max(0, min(16, (504 + 512 * bi - 128 * jb) // 8 + 1))
    return 8 * nvis_jhi


def _tidx(jb, jhi):
    return jb * 16 + jhi if jb < 8 else 128


def _build_program():
    nc = bacc.Bacc("TRN2", target_bir_lowering=False, debug=False, num_devices=8)

    # ---- DRAM I/O ----
    xT = nc.dram_tensor("xT", [DIM, N], F32R, kind="ExternalInput").ap()
    xTq = nc.dram_tensor("xTq", [DIM, NI], F32R, kind="ExternalInput").ap()
    wq = nc.dram_tensor("wq", [DIM, DIM], F32R, kind="ExternalInput").ap()
    wk = nc.dram_tensor("wk", [DIM, DIM], F32R, kind="ExternalInput").ap()
    wv = nc.dram_tensor("wv", [DIM, DIM], F32R, kind="ExternalInput").ap()
    wo = nc.dram_tensor("wo", [DIM, DIM], F32R, kind="ExternalInput").ap()
    memkT = nc.dram_tensor("memkT", [DIM, M], F32R, kind="ExternalInput").ap()
    memv = nc.dram_tensor("memv", [M, DIM], BF16, kind="ExternalInput").ap()
    bdpre = nc.dram_tensor("bdpre", [P, P], F32R, kind="ExternalInput").ap()
    bdpost = nc.dram_tensor("bdpost", [P, P], BF16, kind="ExternalInput").ap()
    aones = nc.dram_tensor("aones", [P, H], BF16, kind="ExternalInput").ap()
    bbc = nc.dram_tensor("bbc", [H, P], BF16, kind="ExternalInput").ap()
    stripd = nc.dram_tensor("stripd", [P, 4], F32, kind="ExternalInput").ap()
    boutd = nc.dram_tensor("boutd", [1, DIM], F32R, kind="ExternalInput").ap()
    onesd = nc.dram_tensor("onesd", [1, P], F32R, kind="ExternalInput").ap()

    outr = nc.dram_tensor("outr", [NI, DIM], F32, kind="ExternalOutput").ap()
    preT2 = nc.dram_tensor("preT2", [NJ, H, NI], F32R, kind="ExternalOutput").ap()
    expm2 = nc.dram_tensor("expm2", [NJ, H, NI], F32, kind="ExternalOutput").ap()
    deno = nc.dram_tensor("deno", [H, NI], F32, kind="ExternalOutput").ap()
    attn2d = nc.dram_tensor("attn2d", [NJ, H, NI], BF16).ap()  # internal scratch

    with tile.TileContext(nc, trace_sim=False) as tc:
        res = tc.alloc_tile_pool(name="res", bufs=1)
        bd_pre_sb = res.tile([P, P], F32R)
        bd_post_sb = res.tile([P, P], BF16)
        aones_sb = res.tile([P, H], BF16)
        bbc_sb = res.tile([H, P], BF16)
        strip_sb = res.tile([P, 4], F32)
        boutbc = res.tile([P, 2, 512], F32)
        QT = res.tile([P, 8, NI], F32R)
        KT = res.tile([P, 8, 1032], F32R)
        V = res.tile([P, 9, DIM], BF16)
        nc.sync.dma_start(bd_pre_sb[:], bdpre[:])
        nc.sync.dma_start(bd_post_sb[:], bdpost[:])
        nc.sync.dma_start(aones_sb[:], aones[:])
        nc.sync.dma_start(bbc_sb[:], bbc[:])
        nc.sync.dma_start(strip_sb[:], stripd[:])

        # ================= Phase QKV =================
        with tc.tile_pool(name="qkv", bufs=1) as qkv, \
             tc.tile_pool(name="psq", bufs=4, space="PSUM") as psq:
            ones_sb = qkv.tile([1, P], F32R, tag="ones")
            bout_sb = qkv.tile([1, DIM], F32R, tag="bout")
            nc.sync.dma_start(ones_sb[:], onesd[:])
            nc.sync.dma_start(bout_sb[:], boutd[:])
            for ch in range(2):
                pbb = psq.tile([P, 512], F32, tag="psq")
                nc.tensor.matmul(pbb[:], ones_sb[:], bout_sb[:, 512 * ch:512 * (ch + 1)],
                                 start=True, stop=True)
                nc.vector.tensor_copy(boutbc[:, ch, :], pbb[:])

            xt_sb = qkv.tile([P, 8, N], F32R, tag="xt")
            xtq_sb = qkv.tile([P, 8, NI], F32R, tag="xtq")
            nc.sync.dma_start(xt_sb[:], xT.rearrange("(s p) n -> p s n", p=P))
            nc.sync.dma_start(xtq_sb[:], xTq.rearrange("(s p) n -> p s n", p=P))

            # Q
            w_sb = qkv.tile([P, 8, DIM], F32R, tag="w")
            nc.sync.dma_start(w_sb[:], wq.rearrange("(s p) n -> p s n", p=P))
            for s in range(8):
                ps = psq.tile([P, 512], F32, tag="psq")
                for sub in range(8):
                    nc.tensor.matmul(ps[:], w_sb[:, sub, 128 * s:128 * (s + 1)],
                                     xtq_sb[:, sub, :],
                                     start=(sub == 0), stop=(sub == 7))
                nc.vector.tensor_copy(QT[:, s, :], ps[:])
            # K
            w_sb = qkv.tile([P, 8, DIM], F32R, tag="w")
            nc.sync.dma_start(w_sb[:], wk.rearrange("(s p) n -> p s n", p=P))
            for s in range(8):
                for ch in range(2):
                    ps = psq.tile([P, 512], F32, tag="psq")
                    for sub in range(8):
                        nc.tensor.matmul(ps[:], w_sb[:, sub, 128 * s:128 * (s + 1)],
                                         xt_sb[:, sub, 512 * ch:512 * (ch + 1)],
                                         start=(sub == 0), stop=(sub == 7))
                    nc.vector.tensor_copy(KT[:, s, 512 * ch:512 * (ch + 1)], ps[:])
            nc.sync.dma_start(KT[:, :, 1024:1028],
                              memkT.rearrange("(s p) m -> p s m", p=P))
            # V
            w_sb = qkv.tile([P, 8, DIM], F32R, tag="w")
            nc.sync.dma_start(w_sb[:], wv.rearrange("(s p) n -> p s n", p=P))
            for jb in range(8):
                for ch in range(2):
                    ps = psq.tile([P, 512], F32, tag="psq")
                    for sub in range(8):
                        nc.tensor.matmul(ps[:], xt_sb[:, sub, 128 * jb:128 * (jb + 1)],
                                         w_sb[:, sub, 512 * ch:512 * (ch + 1)],
                                         start=(sub == 0), stop=(sub == 7))
                    nc.vector.tensor_copy(V[:, jb, 512 * ch:512 * (ch + 1)], ps[:])
            nc.sync.dma_start(V[0:4, 8, :], memv[:])

        # ================= Attention, per i-block pass =================
        expbp = tc.alloc_tile_pool(name="expbp", bufs=1)

        for bi in range(2):
            ic = slice(256 * bi, 256 * (bi + 1))
            vis = []
            for jb in range(9):
                for jhi in range(16 if jb < 8 else 1):
                    kind, c0, cH = _classify(jb, jhi, bi)
                    if kind != "FM":
                        vis.append((jb, jhi, kind, c0, cH))
            expb = expbp.tile([P, 129, 256], BF16, tag="expb")

            # ---- Phase A: dots -> premap -> shuffle -> mix1 -> exp -> den ----
            with tc.tile_pool(name="dotsp", bufs=1) as dotsp, \
                 tc.tile_pool(name="mixp", bufs=1) as mixp, \
                 tc.tile_pool(name="stgp", bufs=4) as stgp, \
                 tc.tile_pool(name="psd", bufs=3, space="PSUM") as psd, \
                 tc.tile_pool(name="psm", bufs=2, space="PSUM") as psm, \
                 tc.tile_pool(name="psdn", bufs=1, space="PSUM") as psdn:
                den_ps = psdn.tile([H, 256], F32, tag="den")
                ndenmm = len(vis)
                idm = 0
                for jb in range(9):
                    jw = 128 if jb < 8 else 4
                    j0 = 128 * jb if jb < 8 else 1024
                    jsl = slice(j0, j0 + jw)
                    dotsT = dotsp.tile([P, H, 256], F32R, tag="dotsT")
                    for h in range(16):
                        s, hh = divmod(h, 2)
                        hp = slice(64 * hh, 64 * hh + 64)
                        kcols = slice(128 * jb, 128 * jb + 128) if jb < 8 \
                            else slice(1024, 1028)
                        ps = psd.tile([P, 256], F32, tag="psd")
                        nc.tensor.matmul(ps[:jw], KT[hp, s, kcols], QT[hp, s, ic],
                                         start=True, stop=True)
                        nc.vector.tensor_copy(dotsT[:jw, h, :], ps[:jw])
                    nc.sync.dma_start(preT2[jsl, :, ic], dotsT[:jw])

                    tiles = [t for t in vis if t[0] == jb]
                    if not tiles:
                        continue
                    mixt = mixp.tile([P, 16, 256], F32R, tag="mixt")
                    if jb < 8:
                        src = preT2[jsl, :, ic].rearrange(
                            "(jhi jl) h i -> (jl h) jhi i", jl=8)
                        nc.sync.dma_start(mixt[:], src)
                    else:
                        src = preT2[jsl, :, ic].rearrange("jl h i -> (jl h) i")
                        nc.sync.dma_start(mixt[:64, 0, :], src)
                    for (_, jhi, kind, c0, cH) in tiles:
                        pw = 128 if jb < 8 else 64
                        ti = _tidx(jb, jhi)
                        jr = slice(j0 + 8 * jhi, j0 + 8 * jhi + 8) if jb < 8 \
                            else slice(1024, 1028)
                        pm = psm.tile([P, 256], F32, tag="psm")
                        nc.tensor.matmul(pm[:pw], bd_pre_sb[:pw, :pw],
                                         mixt[:pw, jhi, :], start=True, stop=True)
                        if kind == "DG" and cH > c0:
                            nc.vector.tensor_tensor(pm[:pw, c0:cH], pm[:pw, c0:cH],
                                                    strip_sb[:pw, 0:cH - c0], ALU.add)
                        c0 = c0 if kind == "DG" else 0
                        stg = stgp.tile([P, 256], F32, tag="stg")
                        nc.scalar.activation(stg[:pw, c0:], pm[:pw, c0:], AF.Exp)
                        nc.sync.dma_start(
                            expm2[jr, :, 256 * bi + c0:256 * (bi + 1)].rearrange(
                                "j h i -> (j h) i"),
                            stg[:pw, c0:])
                        nc.vector.tensor_copy(expb[:pw, ti, c0:], stg[:pw, c0:])
                        if c0 > 0:
                            nc.vector.memset(expb[:pw, ti, 0:c0], 0.0)
                        nc.tensor.matmul(den_ps[:], aones_sb[:pw, :],
                                         expb[:pw, ti, :],
                                         start=(idm == 0), stop=(idm == ndenmm - 1),
                                         skip_group_check=True)
                        idm += 1
                den_sb = stgp.tile([H, 256], F32, tag="densb")
                nc.vector.tensor_copy(den_sb[:], den_ps[:])
                nc.sync.dma_start(deno[:, ic], den_sb[:])
                rec_sb = stgp.tile([H, 256], F32, tag="rec")
                nc.vector.reciprocal(rec_sb[:], den_sb[:])
                rec_bf = stgp.tile([H, 256], BF16, tag="recbf")
                nc.vector.tensor_copy(rec_bf[:], rec_sb[:])

            # ---- Phase B: div -> mix2 -> attn2d -> attn@V ----
            with tc.tile_pool(name="wrkB", bufs=3) as wrkB, \
                 tc.tile_pool(name="rbcp", bufs=1) as rbcp, \
                 tc.tile_pool(name="atp", bufs=1) as atp, \
                 tc.tile_pool(name="psb", bufs=2, space="PSUM") as psb, \
                 tc.tile_pool(name="psov", bufs=2, space="PSUM") as psov:
                AT = atp.tile([P, 8, 256], F32R, tag="AT")
                pbc = psb.tile([P, 256], F32, tag="psb")
                nc.tensor.matmul(pbc[:], bbc_sb[:], rec_bf[:], start=True, stop=True)
                rbc = rbcp.tile([P, 256], BF16, tag="rbc")
                nc.vector.tensor_copy(rbc[:], pbc[:])
                for (jb, jhi, kind, c0, cH) in vis:
                    pw = 128 if jb < 8 else 64
                    ti = _tidx(jb, jhi)
                    j0 = 128 * jb if jb < 8 else 1024
                    jr = slice(j0 + 8 * jhi, j0 + 8 * jhi + 8) if jb < 8 \
                        else slice(1024, 1028)
                    nc.vector.tensor_tensor(expb[:pw, ti, :], expb[:pw, ti, :],
                                            rbc[:pw, :], ALU.mult)
                    pm2 = psb.tile([P, 256], F32, tag="psb")
                    nc.tensor.matmul(pm2[:pw], bd_post_sb[:pw, :pw],
                                     expb[:pw, ti, :], start=True, stop=True)
                    a2 = wrkB.tile([P, 256], BF16, tag="a2")
                    nc.scalar.copy(a2[:pw], pm2[:pw])
                    nc.sync.dma_start(
                        attn2d[jr, :, ic].rearrange("j h i -> (j h) i"), a2[:pw])
                # attn @ V  (O^T accumulation, head pairs share one PSUM tile)
                for t in range(8):
                    pso = psov.tile([P, 256], F32, tag="psov")
                    for hh in range(2):
                        h = 2 * t + hh
                        vjbs = [jb for jb in range(9) if _vis_rows(jb, bi) > 0]
                        for k, jb in enumerate(vjbs):
                            nr = _vis_rows(jb, bi)
                            j0 = 128 * jb if jb < 8 else 1024
                            a2t = wrkB.tile([P, 256], BF16, tag="a2t")
                            nc.sync.dma_start(a2t[:nr, :],
                                              attn2d[j0:j0 + nr, h, ic])
                            nc.tensor.matmul(pso[64 * hh:64 * hh + 64, :],
                                             V[:nr, jb, 64 * h:64 * h + 64],
                                             a2t[:nr, :],
                                             start=(k == 0), stop=(k == len(vjbs) - 1),
                                             skip_group_check=True)
                    nc.vector.tensor_copy(AT[:, t, :], pso[:])

                # ---- Phase C: out projection ----
                with tc.tile_pool(name="wop", bufs=1) as wop, \
                     tc.tile_pool(name="outp", bufs=2) as outp, \
                     tc.tile_pool(name="pso2", bufs=1, space="PSUM") as pso2:
                    pouts = {}
                    for half in range(2):
                        wo_sb = wop.tile([P, 4, DIM], F32R, tag="wo")
                        nc.sync.dma_start(
                            wo_sb[:],
                            wo[512 * half:512 * (half + 1), :].rearrange(
                                "(s p) n -> p s n", p=P))
                        for icb in range(2):
                            for ch in range(2):
                                if half == 0:
                                    po_t = pso2.tile(
                                        [P, 512], F32, tag=f"po{icb}{ch}",
                                        name=f"po{icb}{ch}")
                                    pouts[(icb, ch)] = po_t
                                po = pouts[(icb, ch)]
                                for sl in range(4):
                                    s = 4 * half + sl
                                    nc.tensor.matmul(
                                        po[:],
                                        AT[:, s, 128 * icb:128 * (icb + 1)],
                                        wo_sb[:, sl, 512 * ch:512 * (ch + 1)],
                                        start=(s == 0), stop=(s == 7),
                                        skip_group_check=True)
                    for icb in range(2):
                        for ch in range(2):
                            osb = outp.tile([P, 512], F32, tag="osb")
                            nc.vector.tensor_tensor(osb[:], pouts[(icb, ch)][:],
                                                    boutbc[:, ch, :], ALU.add)
                            nc.sync.dma_start(
                                outr[256 * bi + 128 * icb:256 * bi + 128 * (icb + 1),
                                     512 * ch:512 * (ch + 1)],
                                osb[:])
        expbp.release()
        res.release()
    nc.compile()
    return nc


_PROG = None


def _get_program():
    global _PROG
    if _PROG is None:
        _PROG = _build_program()
    return _PROG


def kernel(x=None, Wq=None, Wk=None, Wv=None, mem_k=None, mem_v=None,
           pre_proj=None, post_proj=None, Wout=None, bout=None):
    nc = _get_program()
    f32 = np.float32
    scale = f32(D) ** f32(-0.5)

    wq_s = np.ascontiguousarray((np.asarray(Wq, f32) * scale))
    wk_s = np.ascontiguousarray(np.asarray(Wk, f32))
    wv_s = np.ascontiguousarray(np.asarray(Wv, f32))
    wo_s = np.ascontiguousarray(np.asarray(Wout, f32))
    memkT = np.ascontiguousarray(
        np.asarray(mem_k, f32).transpose(0, 2, 1).reshape(DIM, M))
    memv_n = np.ascontiguousarray(
        np.asarray(mem_v, f32).transpose(1, 0, 2).reshape(M, DIM)).astype(
            ml_dtypes.bfloat16)
    bd_pre = np.kron(np.eye(8, dtype=f32), np.asarray(pre_proj, f32))
    bd_post = np.kron(np.eye(8, dtype=f32), np.asarray(post_proj, f32)).astype(
        ml_dtypes.bfloat16)
    a_ones = np.tile(np.eye(H, dtype=f32), (8, 1)).astype(ml_dtypes.bfloat16)
    b_bc = np.tile(np.eye(H, dtype=f32), (1, 8)).astype(ml_dtypes.bfloat16)
    bout_r = np.ascontiguousarray(np.asarray(bout, f32).reshape(1, DIM))
    ones_r = np.ones((1, P), f32)

    x = np.asarray(x, f32)
    in_maps = []
    for c in range(8):
        bb, r = divmod(c, 2)
        xt = np.ascontiguousarray(x[bb].T)
        xtq = np.ascontiguousarray(x[bb].T[:, r::2])
        strip = np.zeros((P, 4), f32)
        for p in range(P):
            for xx in range(4):
                if 2 * xx < (p // 16) - r:
                    strip[p, xx] = NEG
        in_maps.append({
            "xT": xt, "xTq": xtq, "wq": wq_s, "wk": wk_s, "wv": wv_s,
            "wo": wo_s, "memkT": memkT, "memv": memv_n, "bdpre": bd_pre,
            "bdpost": bd_post, "aones": a_ones, "bbc": b_bc, "stripd": strip,
            "boutd": bout_r, "onesd": ones_r,
        })

    res = run_bass_kernel_spmd(nc, in_maps, list(range(8)))

    out = np.empty((B, N, DIM), f32)
    pre = np.empty((B, H, N, NJ), f32)
    post = np.empty((B, H, N, NJ), f32)
    for c in range(8):
        bb, r = divmod(c, 8 // B)
        rr = res.results[c]
        out[bb, r::2] = rr["outr"]
        p2 = rr["preT2"].transpose(1, 2, 0)       # [H, NI, NJ]
        pre[bb, :, r::2, M:] = p2[:, :, :N]
        pre[bb, :, r::2, :M] = p2[:, :, N:]
        po = rr["expm2"].transpose(1, 2, 0) / rr["den" "o"][:, :, None]
        post[bb, :, r::2, M:] = po[:, :, :N]
        post[bb, :, r::2, :M] = po[:, :, N:]
    return out, pre, post
